# revision 4
# baseline (speedup 1.0000x reference)
"""RNN-T JointNetwork kernel for Trainium2 (8 NeuronCores).

out[b,t,u,:] = tanh(trans[b,t,:] + pred[b,u,:]) @ W + bias
shapes: trans (4,512,512) pred (4,100,512) W (512,1025) b (1025,)
out (4,512,100,1025) fp32; seq_len passed through.

Sharding: 8 shards = (b, t-half). Each core computes a contiguous
(256,100,1025) output slab.

Per-core pipeline:
  - transpose trans/pred slabs to [H-on-partitions] layout via PE transpose
  - jointT[h, u*256+t] = tanh(transT[h,t] + predT[h,u]) on ScalarE
    (fused add+tanh: per-partition bias = predT column, fp16 out)
  - PE matmul fp16: stationary = jointT 128x128 tile, moving = W16,
    K=512 as 4 accumulating chunks, N=1025 as 512/512/1 into one PSUM tile
  - DVE evicts PSUM + bias -> SBUF fp32, DMA 1.05MB slabs to HBM
"""

import os
import sys

sys.path.insert(0, "/opt/trn_rl_repo")

import numpy as np

import concourse.bass as bass
import concourse.bacc as bacc
import concourse.mybir as mybir
import concourse.tile as tile
from concourse.bass_utils import run_bass_kernel_spmd
from concourse.masks import make_identity


def _install_ntff_hook_shim():
    """bass_utils trace=True needs antenv.axon_hooks, which this image
    lacks; provide it (ctypes against the baked libaxon_pjrt.so)."""
    try:
        import antenv.axon_hooks  # noqa: F401

        return
    except ImportError:
        pass
    import contextlib
    import ctypes
    import types

    so_path = os.environ.get("AXON_PJRT_SO", "/opt/axon/libaxon_pjrt.so")

    def _build_hook():
        if not os.path.exists(so_path):
            return None
        lib = ctypes.CDLL(so_path)
        if not hasattr(lib, "axon_start_nrt_profile"):
            return None
        lib.axon_start_nrt_profile.argtypes = [
            ctypes.POINTER(ctypes.c_int64),
            ctypes.c_size_t,
        ]
        lib.axon_start_nrt_profile.restype = ctypes.c_int64
        lib.axon_stop_nrt_profile.argtypes = [ctypes.c_char_p]
        lib.axon_stop_nrt_profile.restype = ctypes.c_int64

        @contextlib.contextmanager
        def _hook(output_dir, device_ids):
            import jax

            jax.devices()
            if device_ids:
                ids = (ctypes.c_int64 * len(device_ids))(*device_ids)
                rc = lib.axon_start_nrt_profile(ids, len(device_ids))
            else:
                rc = lib.axon_start_nrt_profile(None, 0)
            if rc != 0:
                raise RuntimeError(f"axon_start_nrt_profile rc={rc}")
            try:
                yield
            finally:
                n = lib.axon_stop_nrt_profile(str(output_dir).encode())
                print(f"profile: {n} file(s) -> {output_dir}", file=sys.stderr)

        return _hook

    mod = types.ModuleType("antenv.axon_hooks")
    _state = {"hook": None, "tried": False}

    def get_axon_ntff_profile_hook():
        if not _state["tried"]:
            _state["tried"] = True
            _state["hook"] = _build_hook()
        return _state["hook"]

    def set_axon_ntff_profile_hook(hook):
        _state["hook"] = hook
        _state["tried"] = True

    mod.get_axon_ntff_profile_hook = get_axon_ntff_profile_hook
    mod.set_axon_ntff_profile_hook = set_axon_ntff_profile_hook
    sys.modules["antenv.axon_hooks"] = mod
    try:
        import antenv

        antenv.axon_hooks = mod
    except ImportError:
        pass


_install_ntff_hook_shim()

F32 = mybir.dt.float32
F16 = mybir.dt.float16

B, T, U, H, V1 = 4, 512, 100, 512, 1025
T_CORE = T // 2  # 256 t-rows per core
KC = H // 128  # 4 contraction chunks
GU = 10  # u's per group
N_GROUPS = U // GU
TILES_PER_GROUP = GU * T_CORE // 128  # 20
N_CHUNKS = [(0, 512), (512, 1024), (1024, 1025)]


def build_nc(joint_bufs=2, out_bufs=3, psum_bufs=2):
    nc = bacc.Bacc("TRN2", target_bir_lowering=False, debug=False, num_devices=8)
    trans_d = nc.dram_tensor("trans", [T_CORE, H], F32, kind="ExternalInput").ap()
    pred_d = nc.dram_tensor("pred", [U, H], F32, kind="ExternalInput").ap()
    w_d = nc.dram_tensor("W", [H, V1], F32, kind="ExternalInput").ap()
    b_d = nc.dram_tensor("b", [V1], F32, kind="ExternalInput").ap()
    out_d = nc.dram_tensor("out", [T_CORE, U, V1], F32, kind="ExternalOutput").ap()

    # per-u destination view: out_view[u] = [p(t-low) 128, g(t-high) 2, v]
    out_view = out_d.rearrange("(g p) u v -> u p g v", p=128)

    with tile.TileContext(nc) as tc:
        with (
            tc.tile_pool(name="const", bufs=1) as const,
            tc.tile_pool(name="stage", bufs=2) as stage,
            tc.tile_pool(name="joint", bufs=joint_bufs) as joint_pool,
            tc.tile_pool(name="outs", bufs=out_bufs) as out_pool,
            tc.tile_pool(name="psum", bufs=psum_bufs, space="PSUM") as psum_pool,
            tc.tile_pool(name="psumtr", bufs=2, space="PSUM") as psum_tr,
        ):
            identity = const.tile([128, 128], F32)
            make_identity(nc, identity)

            # bias replicated across partitions (one-time broadcast DMA)
            bias_rep = const.tile([128, V1], F32)
            nc.sync.dma_start(out=bias_rep, in_=b_d.partition_broadcast(128))

            # W -> fp16, H on partitions (4 chunks)
            w16 = const.tile([128, KC, V1], F16)
            for c in range(KC):
                wst = stage.tile([128, V1], F32, tag="wst")
                nc.sync.dma_start(out=wst, in_=w_d[c * 128 : (c + 1) * 128, :])
                nc.vector.tensor_copy(w16[:, c, :], wst)

            # transT[h, t] via PE transpose of natural-layout tiles
            trans_t = const.tile([128, KC, T_CORE], F32)
            for th in range(T_CORE // 128):
                tn = stage.tile([128, H], F32, tag="tn")
                nc.sync.dma_start(out=tn, in_=trans_d[th * 128 : (th + 1) * 128, :])
                for c in range(KC):
                    pt = psum_tr.tile([128, 128], F32)
                    nc.tensor.transpose(pt, tn[:, c * 128 : (c + 1) * 128], identity)
                    nc.vector.tensor_copy(
                        trans_t[:, c, th * 128 : (th + 1) * 128], pt
                    )

            # predT[h, u]
            pred_t = const.tile([128, KC, U], F32)
            pn = stage.tile([128, H], F32, tag="tn")
            nc.sync.dma_start(out=pn[:U, :], in_=pred_d)
            for c in range(KC):
                pt = psum_tr.tile([128, 128], F32)
                nc.tensor.transpose(
                    pt[:, :U], pn[:U, c * 128 : (c + 1) * 128], identity[:U, :U]
                )
                nc.vector.tensor_copy(pred_t[:, c, :], pt[:, :U])

            for g in range(N_GROUPS):
                joint_t = joint_pool.tile([128, KC, GU * T_CORE], F16)
                for ul in range(GU):
                    u = g * GU + ul
                    for c in range(KC):
                        nc.scalar.activation(
                            joint_t[:, c, ul * T_CORE : (ul + 1) * T_CORE],
                            trans_t[:, c, :],
                            mybir.ActivationFunctionType.Tanh,
                            bias=pred_t[:, c, u : u + 1],
                            scale=1.0,
                        )
                for ml in range(TILES_PER_GROUP):
                    psum = psum_pool.tile([128, V1], F32)
                    msl = slice(ml * 128, (ml + 1) * 128)
                    for ns, ne in N_CHUNKS:
                        for c in range(KC):
                            nc.tensor.matmul(
                                psum[:, ns:ne],
                                lhsT=joint_t[:, c, msl],
                                rhs=w16[:, c, ns:ne],
                                start=(c == 0),
                                stop=(c == KC - 1),
                            )
                    if ml % 2 == 0:
                        out_stage = out_pool.tile([128, 2, V1], F32)
                    nc.vector.tensor_add(out_stage[:, ml % 2, :], psum[:, :], bias_rep)
                    if ml % 2 == 1:
                        u = g * GU + ml // 2
                        nc.sync.dma_start(out=out_view[u], in_=out_stage)

    nc.compile()
    return nc


_CACHE = {}


def get_nc():
    if "nc" not in _CACHE:
        _CACHE["nc"] = build_nc()
    return _CACHE["nc"]


def shard_inputs(trans, pred, W, b):
    in_maps = []
    for i in range(8):
        bb, th = divmod(i, 2)
        in_maps.append(
            {
                "trans": np.ascontiguousarray(
                    trans[bb, th * T_CORE : (th + 1) * T_CORE]
                ),
                "pred": np.ascontiguousarray(pred[bb]),
                "W": W,
                "b": b,
            }
        )
    return in_maps


def kernel(trans, pred, seq_len, W, b, _trace=False):
    trans = np.ascontiguousarray(np.asarray(trans, dtype=np.float32))
    pred = np.ascontiguousarray(np.asarray(pred, dtype=np.float32))
    W = np.ascontiguousarray(np.asarray(W, dtype=np.float32))
    b = np.ascontiguousarray(np.asarray(b, dtype=np.float32))
    seq_len = np.asarray(seq_len, dtype=np.int32)

    nc = get_nc()
    in_maps = shard_inputs(trans, pred, W, b)
    res = run_bass_kernel_spmd(nc, in_maps, core_ids=list(range(8)), trace=_trace)
    out = np.empty((B, T, U, V1), np.float32)
    for i in range(8):
        bb, th = divmod(i, 2)
        out[bb, th * T_CORE : (th + 1) * T_CORE] = res.results[i]["out"]
    if _trace:
        return (out, seq_len), res
    return out, seq_len


# revision 13
# speedup vs baseline: 1.0916x; 1.0916x over previous
"""RNN-T JointNetwork kernel for Trainium2 (8 NeuronCores).

out[b,t,u,:] = tanh(trans[b,t,:] + pred[b,u,:]) @ W + bias
shapes: trans (4,512,512) pred (4,100,512) W (512,1025) b (1025,)
out (4,512,100,1025) fp32; seq_len passed through.

Sharding: 8 shards = (b, t-half). Each core computes a contiguous
(256,100,1025) output slab.

Per-core pipeline:
  - transpose trans/pred slabs to [H-on-partitions] layout via PE transpose
  - jointT[h, u*256+t] = tanh(transT[h,t] + predT[h,u]) on ScalarE
    (fused add+tanh: per-partition bias = predT column, fp16 out)
  - PE matmul fp16: stationary = jointT 128x128 tile, moving = W16,
    K=512 as 4 accumulating chunks, N=1025 as 512/512/1 into one PSUM tile
  - DVE evicts PSUM + bias -> SBUF fp32, DMA 1.05MB slabs to HBM
"""

import os
import sys

sys.path.insert(0, "/opt/trn_rl_repo")

import numpy as np

import concourse.bass as bass
import concourse.bacc as bacc
import concourse.mybir as mybir
import concourse.tile as tile
from concourse.bass_utils import run_bass_kernel_spmd
from concourse.masks import make_identity


def _install_ntff_hook_shim():
    """bass_utils trace=True needs antenv.axon_hooks, which this image
    lacks; provide it (ctypes against the baked libaxon_pjrt.so)."""
    try:
        import antenv.axon_hooks  # noqa: F401

        return
    except ImportError:
        pass
    import contextlib
    import ctypes
    import types

    so_path = os.environ.get("AXON_PJRT_SO", "/opt/axon/libaxon_pjrt.so")

    def _build_hook():
        if not os.path.exists(so_path):
            return None
        lib = ctypes.CDLL(so_path)
        if not hasattr(lib, "axon_start_nrt_profile"):
            return None
        lib.axon_start_nrt_profile.argtypes = [
            ctypes.POINTER(ctypes.c_int64),
            ctypes.c_size_t,
        ]
        lib.axon_start_nrt_profile.restype = ctypes.c_int64
        lib.axon_stop_nrt_profile.argtypes = [ctypes.c_char_p]
        lib.axon_stop_nrt_profile.restype = ctypes.c_int64

        @contextlib.contextmanager
        def _hook(output_dir, device_ids):
            import jax

            jax.devices()
            if device_ids:
                ids = (ctypes.c_int64 * len(device_ids))(*device_ids)
                rc = lib.axon_start_nrt_profile(ids, len(device_ids))
            else:
                rc = lib.axon_start_nrt_profile(None, 0)
            if rc != 0:
                raise RuntimeError(f"axon_start_nrt_profile rc={rc}")
            try:
                yield
            finally:
                n = lib.axon_stop_nrt_profile(str(output_dir).encode())
                print(f"profile: {n} file(s) -> {output_dir}", file=sys.stderr)

        return _hook

    mod = types.ModuleType("antenv.axon_hooks")
    _state = {"hook": None, "tried": False}

    def get_axon_ntff_profile_hook():
        if not _state["tried"]:
            _state["tried"] = True
            _state["hook"] = _build_hook()
        return _state["hook"]

    def set_axon_ntff_profile_hook(hook):
        _state["hook"] = hook
        _state["tried"] = True

    mod.get_axon_ntff_profile_hook = get_axon_ntff_profile_hook
    mod.set_axon_ntff_profile_hook = set_axon_ntff_profile_hook
    sys.modules["antenv.axon_hooks"] = mod
    try:
        import antenv

        antenv.axon_hooks = mod
    except ImportError:
        pass


_install_ntff_hook_shim()

F32 = mybir.dt.float32
F16 = mybir.dt.float16

B, T, U, H, V1 = 4, 512, 100, 512, 1025
T_CORE = T // 2  # 256 t-rows per core
KC = H // 128  # 4 contraction chunks
GU = 10  # u's per group
N_GROUPS = U // GU
TILES_PER_GROUP = GU * T_CORE // 128  # 20
N_CHUNKS = [(0, 512), (512, 1024), (1024, 1025)]


def build_nc(joint_bufs=4, out_bufs=6, psum_bufs=3, c_outer=False, warmup_mms=24,
             skip_last_col=False):
    nc = bacc.Bacc("TRN2", target_bir_lowering=False, debug=False, num_devices=8)
    trans_d = nc.dram_tensor("trans", [T_CORE, H], F32, kind="ExternalInput").ap()
    pred_d = nc.dram_tensor("pred", [U, H], F32, kind="ExternalInput").ap()
    w_d = nc.dram_tensor("W", [H, V1], F32, kind="ExternalInput").ap()
    b_d = nc.dram_tensor("b", [V1], F32, kind="ExternalInput").ap()
    out_d = nc.dram_tensor("out", [T_CORE, U, V1], F32, kind="ExternalOutput").ap()

    # per-u destination view: out_view[u] = [p(t-low) 128, g(t-high) 2, v]
    out_view = out_d.rearrange("(g p) u v -> u p g v", p=128)

    n_chunks = N_CHUNKS[:2] if skip_last_col else N_CHUNKS

    with tile.TileContext(nc) as tc:
        with (
            tc.tile_pool(name="const", bufs=1) as const,
            tc.tile_pool(name="stage", bufs=2) as stage,
            tc.tile_pool(name="joint", bufs=joint_bufs) as joint_pool,
            tc.tile_pool(name="outs", bufs=out_bufs) as out_pool,
            tc.tile_pool(name="psum", bufs=psum_bufs, space="PSUM") as psum_pool,
            tc.tile_pool(name="psumb", bufs=2, space="PSUM") as psum_pool_b,
        ):
            # PE warmup: keep HAM busy from t=0 so real matmuls start at 2.4GHz
            if warmup_mms:
                wz = const.tile([128, 128], F16)
                nc.vector.memset(wz, 0.0)
                wps = psum_pool_b.tile([128, 128], F32, tag="small")
                for _ in range(warmup_mms):
                    nc.tensor.matmul(wps, lhsT=wz, rhs=wz, start=True, stop=True)

            identity = const.tile([128, 128], F32)
            make_identity(nc, identity)

            # transT[h, t] via PE transpose of natural-layout tiles
            trans_t = const.tile([128, KC, T_CORE], F32)
            for th in range(T_CORE // 128):
                tn = stage.tile([128, H], F32, tag="tn")
                nc.sync.dma_start(out=tn, in_=trans_d[th * 128 : (th + 1) * 128, :])
                for c in range(KC):
                    pt = psum_pool_b.tile([128, 128], F32, tag="small")
                    nc.tensor.transpose(pt, tn[:, c * 128 : (c + 1) * 128], identity)
                    nc.vector.tensor_copy(
                        trans_t[:, c, th * 128 : (th + 1) * 128], pt
                    )

            # predT[h, u]
            pred_t = const.tile([128, KC, U], F32)
            pn = stage.tile([128, H], F32, tag="tn")
            nc.sync.dma_start(out=pn[:U, :], in_=pred_d)
            for c in range(KC):
                pt = psum_pool_b.tile([128, 128], F32, tag="small")
                nc.tensor.transpose(
                    pt[:, :U], pn[:U, c * 128 : (c + 1) * 128], identity[:U, :U]
                )
                nc.vector.tensor_copy(pred_t[:, c, :], pt[:, :U])

            # W -> fp16 directly via SWDGE cast-DMA, H on partitions (4 chunks)
            w16 = const.tile([128, KC, V1], F16)
            for c in range(KC):
                nc.gpsimd.dma_start(
                    out=w16[:, c, :], in_=w_d[c * 128 : (c + 1) * 128, :]
                )

            # bias replicated across partitions (one-time broadcast DMA)
            bias_rep = const.tile([128, V1], F32)
            nc.sync.dma_start(out=bias_rep, in_=b_d.partition_broadcast(128))

            for g in range(N_GROUPS):
                joint_t = joint_pool.tile([128, KC, GU * T_CORE], F16)
                for ul in range(GU):
                    u = g * GU + ul
                    for c in range(KC):
                        nc.scalar.activation(
                            joint_t[:, c, ul * T_CORE : (ul + 1) * T_CORE],
                            trans_t[:, c, :],
                            mybir.ActivationFunctionType.Tanh,
                            bias=pred_t[:, c, u : u + 1],
                            scale=1.0,
                        )
                for ml in range(TILES_PER_GROUP):
                    psum = psum_pool.tile([128, 1024], F32)
                    msl = slice(ml * 128, (ml + 1) * 128)
                    if c_outer:
                        mm_order = [(c, nch) for c in range(KC) for nch in n_chunks]
                    else:
                        mm_order = [(c, nch) for nch in n_chunks for c in range(KC)]
                    for c, (ns, ne) in mm_order:
                        if ne <= 1024:
                            nc.tensor.matmul(
                                psum[:, ns:ne],
                                lhsT=joint_t[:, c, msl],
                                rhs=w16[:, c, ns:ne],
                                start=(c == 0),
                                stop=(c == KC - 1),
                            )
                        else:
                            if c == 0:
                                psum_b = psum_pool_b.tile([128, 1], F32, tag="small")
                            nc.tensor.matmul(
                                psum_b,
                                lhsT=joint_t[:, c, msl],
                                rhs=w16[:, c, ns:ne],
                                start=(c == 0),
                                stop=(c == KC - 1),
                            )
                    if ml % 2 == 0:
                        out_stage = out_pool.tile([128, 2, V1], F32)
                    nc.vector.tensor_add(
                        out_stage[:, ml % 2, 0:1024], psum[:, :], bias_rep[:, 0:1024]
                    )
                    if not skip_last_col:
                        # 1-col eviction on the (idle) scalar engine:
                        # out = Identity(psum_b * 1 + b[1024])
                        nc.scalar.activation(
                            out_stage[:, ml % 2, 1024:1025],
                            psum_b,
                            mybir.ActivationFunctionType.Identity,
                            bias=bias_rep[:, 1024:1025],
                            scale=1.0,
                        )
                    if ml % 2 == 1:
                        u = g * GU + ml // 2
                        nc.sync.dma_start(out=out_view[u], in_=out_stage)

    nc.compile()
    return nc


_CACHE = {}


def get_nc():
    if "nc" not in _CACHE:
        import json

        opts = json.loads(os.environ.get("JOINT_KERNEL_OPTS", "{}"))
        _CACHE["nc"] = build_nc(**opts)
    return _CACHE["nc"]


def shard_inputs(trans, pred, W, b):
    in_maps = []
    for i in range(8):
        bb, th = divmod(i, 2)
        in_maps.append(
            {
                "trans": np.ascontiguousarray(
                    trans[bb, th * T_CORE : (th + 1) * T_CORE]
                ),
                "pred": np.ascontiguousarray(pred[bb]),
                "W": W,
                "b": b,
            }
        )
    return in_maps


def kernel(trans, pred, seq_len, W, b, _trace=False):
    trans = np.ascontiguousarray(np.asarray(trans, dtype=np.float32))
    pred = np.ascontiguousarray(np.asarray(pred, dtype=np.float32))
    W = np.ascontiguousarray(np.asarray(W, dtype=np.float32))
    b = np.ascontiguousarray(np.asarray(b, dtype=np.float32))
    seq_len = np.asarray(seq_len, dtype=np.int32)

    nc = get_nc()
    in_maps = shard_inputs(trans, pred, W, b)
    res = run_bass_kernel_spmd(nc, in_maps, core_ids=list(range(8)), trace=_trace)
    out = np.empty((B, T, U, V1), np.float32)
    for i in range(8):
        bb, th = divmod(i, 2)
        out[bb, th * T_CORE : (th + 1) * T_CORE] = res.results[i]["out"]
    if _trace:
        return (out, seq_len), res
    return out, seq_len
